# revision 27
# baseline (speedup 1.0000x reference)
"""Llama3 GQA decode attention (B=8, q_len=1, past=4096) on 8 TRN2 cores.

Sharding: tensor-parallel over heads. Core c owns q-heads [4c, 4c+4) and
kv-head c: Wq/Wk/Wv output-dim sharded, Wo input-dim sharded, KV cache
sharded by kv head. Each core computes a partial o_proj output [8, 4096];
the host sum over cores is the all-reduce.

The kernel is HBM-bandwidth bound (per-core working set = KV cache slice +
weight slices), so bytes are minimized: the KV cache is stored int8 with
per-(batch,d) scales for K and per-(batch, s mod 128) scales for V,
dequantized to fp16 on-device (K on DVE, V on ACT, overlapped with DMA);
weights and activations are fp16; matmuls run fp16 x fp16 -> fp32 PSUM.
int8 weights were tried and fail the 2e-2 gate (2.2e-2); fp8 KV also fails
(3.7e-2). Measured end-to-end rel err ~1.45e-2.

Device layouts are partition-major so every big DMA moves >=2KB contiguous
per partition (sub-512B segments cost 2x). Host prepares (data movement +
quantize only):
  xR    [128, 32, 8]      x.T tiled:   xR[p, t, b] = x[b, t*128+p]
  wR    [128, 32, 768]    wqkvT tiled (fp16)
  woT   [512, 4096]       Wo[:, 512c:512c+512].T (fp16)
  kT8   [8, 128, 4096]    past_k[:, c] d-major int8, scale per (b, d)
  vR8   [8, 128, 32, 128] past_v[:, c] tiled int8: vR8[b, p, t, d] =
                          v[b, t*128+p, d], scale per (b, p)
  cst   [128, 21]         ropes(4) | ones(1) | kscale[b](8) | vscale[b](8)
All matmuls contract over the partition dim; no large transpose is ever
needed on device.
"""

import sys

sys.path.insert(0, "/opt/trn_rl_repo")

import numpy as np
import ml_dtypes

import concourse.bacc as bacc
import concourse.tile as tile
from concourse import mybir
from concourse.bass_utils import run_bass_kernel_spmd

B = 8            # batch
NH = 32          # query heads total
NKV = 8          # kv heads total
D = 128          # head dim
HID = 4096       # hidden
S = 4096         # past length
NCORES = 8
HQ = NH // NCORES          # 4 query heads per core
QKV = HQ * D + 2 * D       # 768 projection outputs per core
T = S // 128               # 32 seq tiles
ROPE_THETA = 500000.0

F32 = mybir.dt.float32
F16 = mybir.dt.float16
I8 = mybir.dt.int8
EXP = mybir.ActivationFunctionType.Exp
COPY = mybir.ActivationFunctionType.Copy
NPF16 = np.float16

_CACHE = {}

# per-batch dequant engine for K and V: "d"=DVE, "a"=ACT, "p"=Pool.
# Measured on HW: DVE ~2us/op, ACT ~3us/op; Pool is ~50x slower than the
# cost model claims (Q7 software handler) - never use it here.
DEQUANT_K = ["d"] * 8
DEQUANT_V = ["a"] * 8


def _load_batch(nc, b, h, cst_sb, kv8_pool, kv_pool):
    """Emit the int8 KV loads + dequant for batch b; returns (kt_b, v_b)."""
    k8_b = kv8_pool.tile([128, S], I8, tag="k8")
    v8_b = kv8_pool.tile([128, T, D], I8, tag="v8")
    CH = S // 2
    CHT = T // 2
    for ci in range(2):
        nc.sync.dma_start(
            out=k8_b[:, ci * CH:(ci + 1) * CH],
            in_=h["kT8"][b][:, ci * CH:(ci + 1) * CH])
        nc.sync.dma_start(
            out=v8_b[:, ci * CHT:(ci + 1) * CHT, :],
            in_=h["vR8"][b][:, ci * CHT:(ci + 1) * CHT, :])

    # dequant to fp16; engine assignment per batch is tunable: "d" = DVE
    # (fastest), "a" = ACT, "p" = Pool (slowest, Q7 software handler)
    kt_b = kv_pool.tile([128, S], F16, tag="kt")
    v_b = kv_pool.tile([128, T, D], F16, tag="v")
    ksc = cst_sb[:, 5 + b:6 + b]
    vsc = cst_sb[:, 13 + b:14 + b]
    v_flat_out = v_b.rearrange("p t d -> p (t d)")
    v_flat_in = v8_b.rearrange("p t d -> p (t d)")
    HV = T * D // 2
    jobs = [(DEQUANT_K[b], kt_b, k8_b, ksc)]
    if b >= B - 2:
        # tail batches: split V across ACT+DVE halves to cut the serial
        # chain before phase C (V ready ~0.7us earlier)
        jobs.append(("a", v_flat_out[:, 0:HV], v_flat_in[:, 0:HV], vsc))
        jobs.append(("d", v_flat_out[:, HV:], v_flat_in[:, HV:], vsc))
    else:
        jobs.append((DEQUANT_V[b], v_flat_out, v_flat_in, vsc))
    for eng, out_, in_, sc in jobs:
        if eng == "d":
            nc.vector.tensor_scalar_mul(out_, in_, sc)
        elif eng == "p":
            nc.gpsimd.tensor_scalar_mul(out_, in_, sc)
        else:
            nc.scalar.activation(out=out_, in_=in_, func=COPY, scale=sc)
    return kt_b, v_b


def _body(nc, tc, ctx, h):
    """Emit one full forward pass. h = dict of dram handles."""
    consts = ctx.enter_context(tc.tile_pool(name="consts", bufs=1))
    kv8_pool = ctx.enter_context(tc.tile_pool(name="kv8", bufs=3))
    kv_pool = ctx.enter_context(tc.tile_pool(name="kv", bufs=4))
    exp_pool = ctx.enter_context(tc.tile_pool(name="exp", bufs=2))
    small = ctx.enter_context(tc.tile_pool(name="small", bufs=2))
    wo_pool = ctx.enter_context(tc.tile_pool(name="wo", bufs=1))
    osb_pool = ctx.enter_context(tc.tile_pool(name="osb", bufs=1))

    # ---- constants / persistent SBUF ----
    # DMA issue order matters (the cost model completes DMAs in issue
    # order): first w chunk ramps the engines, consts follow (the first
    # dequant needs cst), then w chunks interleave with the first batches'
    # KV loads so dequant work starts early and spreads over the stream
    w_sb = consts.tile([128, T, QKV], F16)
    WCH = T // 4
    nc.sync.dma_start(out=w_sb[:, 0:WCH, :], in_=h["wR"][:, 0:WCH, :])
    x_sb = consts.tile([128, T, B], F16)
    nc.sync.dma_start(out=x_sb, in_=h["xR"][:, :, :])
    cst_sb = consts.tile([128, 21], F32)
    nc.sync.dma_start(out=cst_sb, in_=h["cst"][:, :])
    onesr_sb = consts.tile([1, D], F32)
    nc.sync.dma_start(out=onesr_sb, in_=h["onesr"][:, :])

    batch_tiles = {}
    for ci in range(1, 4):
        if ci <= 2:
            batch_tiles[ci - 1] = _load_batch(nc, ci - 1, h, cst_sb,
                                              kv8_pool, kv_pool)
        sl_ = slice(ci * WCH, (ci + 1) * WCH)
        nc.sync.dma_start(out=w_sb[:, sl_, :], in_=h["wR"][:, sl_, :])

    qT_sb = consts.tile([D, HQ, B], F16)    # roped qT, scaled by 1/sqrt(D)
    kTn_sb = consts.tile([D, B], F16)       # roped new-k (kT layout)
    vT_sb = consts.tile([D, B], F32)         # new v, transposed layout
    oT_sb = consts.tile([D, HQ, B], F16)    # normalized attn out, oT layout

    qcos = cst_sb[:, 0:1]
    qsin = cst_sb[:, 1:2]
    kcos = cst_sb[:, 2:3]
    ksin = cst_sb[:, 3:4]
    ones_sb = cst_sb[:, 4:5]

    # ---- phase A: QKV projections (weights stationary) ----
    with tc.tile_pool(name="psA", bufs=6, space="PSUM") as psA:
        pj = [psA.tile([D, B], F32, tag="pj", name=f"pj{j}", bufs=6)
              for j in range(HQ + 2)]
        for t in range(T):
            for j in range(HQ + 2):
                nc.tensor.matmul(
                    pj[j], w_sb[:, t, j * D:(j + 1) * D], x_sb[:, t, :],
                    start=(t == 0), stop=(t == T - 1),
                )

        # RoPE on qT (per-partition cos/sin) + scale baked into consts
        for q in range(HQ):
            shuf = small.tile([D, B], F32, tag="shuf")
            nc.vector.tensor_copy(shuf[0:64, :], pj[q][64:128, :])
            nc.vector.tensor_copy(shuf[64:128, :], pj[q][0:64, :])
            nc.vector.tensor_scalar_mul(shuf, shuf, qsin)
            qf = small.tile([D, B], F32, tag="qf")
            nc.vector.scalar_tensor_tensor(
                out=qf, in0=pj[q], scalar=qcos,
                in1=shuf, op0=mybir.AluOpType.mult, op1=mybir.AluOpType.add,
            )
            nc.vector.tensor_copy(qT_sb[:, q, :], qf)    # cast to bf16
        # RoPE on new k
        shufk = small.tile([D, B], F32, tag="shuf")
        nc.vector.tensor_copy(shufk[0:64, :], pj[HQ][64:128, :])
        nc.vector.tensor_copy(shufk[64:128, :], pj[HQ][0:64, :])
        nc.vector.tensor_scalar_mul(shufk, shufk, ksin)
        kf = small.tile([D, B], F32, tag="qf")
        nc.vector.scalar_tensor_tensor(
            out=kf, in0=pj[HQ], scalar=kcos,
            in1=shufk, op0=mybir.AluOpType.mult, op1=mybir.AluOpType.add,
        )
        nc.vector.tensor_copy(kTn_sb, kf)                # cast to bf16
        # new v stays fp32 in transposed layout [d, b]
        nc.vector.tensor_copy(vT_sb, pj[HQ + 1])

    # ---- phase B: attention per batch ----
    with tc.tile_pool(name="psST", bufs=2, space="PSUM") as psST, \
         tc.tile_pool(name="psOT", bufs=2, space="PSUM") as psOT, \
         tc.tile_pool(name="psSL", bufs=1, space="PSUM") as psSL, \
         tc.tile_pool(name="psZ", bufs=1, space="PSUM") as psZ, \
         tc.tile_pool(name="psZB", bufs=1, space="PSUM") as psZB:
        for b in range(B):
            if b in batch_tiles:
                kt_b, v_b = batch_tiles[b]
            else:
                kt_b, v_b = _load_batch(nc, b, h, cst_sb, kv8_pool, kv_pool)

            kt_v = kt_b.rearrange("p (t s) -> p t s", s=128)
            # all 32 score tiles accumulate into one PSUM tile -> one exp
            st = psST.tile([128, T * HQ], F32)
            for t in range(T):
                nc.tensor.matmul(st[:, t * HQ:(t + 1) * HQ], kt_v[:, t, :],
                                 qT_sb[:, :, b], start=True, stop=True)
            exp_sb = exp_pool.tile([128, T, HQ], F16)
            nc.scalar.activation(
                out=exp_sb.rearrange("p t h -> p (t h)"), in_=st, func=EXP)

            oT_ps = psOT.tile([D, HQ], F32)
            for t in range(T):
                nc.tensor.matmul(oT_ps, v_b[:, t, :], exp_sb[:, t, :],
                                 start=(t == 0), stop=(t == T - 1))
            # current position (no mask needed: q_pos >= all k_pos)
            sl = psSL.tile([1, HQ], F32)
            nc.tensor.matmul(sl, kTn_sb[:, b:b + 1], qT_sb[:, :, b],
                             start=True, stop=True)
            expl = small.tile([1, HQ], F32, tag="expl")
            nc.scalar.activation(out=expl, in_=sl, func=EXP)
            # rank-1 update v_new[d] * expl[h], via broadcast matmul + DVE
            eb_ps = psZB.tile([128, HQ], F32, tag="eb")
            nc.tensor.matmul(eb_ps, onesr_sb, expl, start=True, stop=True)
            vl_sb = small.tile([128, HQ], F32, tag="vl")
            nc.vector.tensor_scalar_mul(vl_sb, eb_ps, vT_sb[:, b:b + 1])

            # softmax denominator Z = sum(exp)  (partition+tile sum)
            zpart = small.tile([128, HQ], F32, tag="zpart")
            nc.vector.reduce_sum(
                out=zpart, in_=exp_sb.rearrange("p t h -> p h t"),
                axis=mybir.AxisListType.X)
            z_ps = psZ.tile([1, HQ], F32)
            nc.tensor.matmul(z_ps, ones_sb, zpart, start=True, stop=False)
            nc.tensor.matmul(z_ps, ones_sb[0:1, :], expl,
                             start=False, stop=True)
            rz = small.tile([1, HQ], F32, tag="rz")
            nc.vector.reciprocal(rz, z_ps)
            zb_ps = psZB.tile([128, HQ], F32, tag="zb")
            nc.tensor.matmul(zb_ps, onesr_sb, rz, start=True, stop=True)
            zb_sb = small.tile([128, HQ], F32, tag="zbs")
            nc.vector.tensor_copy(zb_sb, zb_ps)
            # (cache PV + new-token term), normalize, scatter to [d, h, b]
            s1_sb = small.tile([128, HQ], F32, tag="s1")
            nc.vector.tensor_add(s1_sb, oT_ps, vl_sb)
            nc.vector.tensor_mul(oT_sb[:, :, b], s1_sb, zb_sb)

    # ---- phase C: o_proj partial ----
    # wo is the last DMA stream; loading it in half-tensor chunks lets the
    # first 4 output chunks' matmuls overlap the second half's transfer
    # (quarter chunks measured no better: HWDGE issue cost eats the gain)
    with tc.tile_pool(name="psO", bufs=4, space="PSUM") as psO:
        wo_sb = []
        for q in range(HQ):
            w = wo_pool.tile([D, HID], F16, tag=f"wo{q}")
            wo_sb.append(w)
        for half in range(2):
            hs = slice(half * (HID // 2), (half + 1) * (HID // 2))
            for q in range(HQ):
                nc.sync.dma_start(
                    out=wo_sb[q][:, hs], in_=h["woT"][q * D:(q + 1) * D, hs])
        o_sb = osb_pool.tile([B, HID], F32)
        for n in range(HID // 512):
            o_ps = psO.tile([B, 512], F32)
            for q in range(HQ):
                nc.tensor.matmul(
                    o_ps, oT_sb[:, q, :], wo_sb[q][:, n * 512:(n + 1) * 512],
                    start=(q == 0), stop=(q == HQ - 1))
            nc.vector.tensor_copy(o_sb[:, n * 512:(n + 1) * 512], o_ps)
        nc.sync.dma_start(out=h["o"][:, :], in_=o_sb)


def _build_module(reps=1):
    nc = bacc.Bacc()
    h = {
        "xR": nc.declare_dram_parameter("xR", [128, T, B], F16, isOutput=False),
        "wR": nc.declare_dram_parameter("wR", [128, T, QKV], F16, isOutput=False),
        "woT": nc.declare_dram_parameter("woT", [HQ * D, HID], F16, isOutput=False),
        "kT8": nc.declare_dram_parameter("kT8", [B, D, S], I8, isOutput=False),
        "vR8": nc.declare_dram_parameter("vR8", [B, 128, T, D], I8, isOutput=False),
        "cst": nc.declare_dram_parameter("cst", [128, 21], F32, isOutput=False),
        "onesr": nc.declare_dram_parameter("onesr", [1, D], F32, isOutput=False),
        "o": nc.declare_dram_parameter("o", [B, HID], F32, isOutput=True),
    }

    with tile.TileContext(nc) as tc:
        from contextlib import ExitStack

        if reps == 1:
            with ExitStack() as ctx:
                _body(nc, tc, ctx, h)
        else:
            with tc.For_i(0, reps, 1):
                with ExitStack() as ctx:
                    _body(nc, tc, ctx, h)

    nc.compile()
    return nc


def _rope_consts():
    inv = ROPE_THETA ** (-np.arange(0, 64, dtype=np.float64) * 2.0 / D)
    ang = float(S) * inv
    cos = np.cos(np.concatenate([ang, ang])).astype(np.float64)
    sin = np.sin(np.concatenate([ang, ang])).astype(np.float64)
    sin_signed = np.concatenate([-sin[:64], sin[64:]])
    scale = 1.0 / np.sqrt(D)
    return np.stack(
        [cos * scale, sin_signed * scale, cos, sin_signed], axis=1
    ).astype(np.float32)                                   # [128, 4]


def _quant8(x, axis):
    amax = np.max(np.abs(x), axis=axis, keepdims=True)
    scale = (amax / 127.0).astype(np.float32)
    scale = np.maximum(scale, 1e-30)
    xi = np.round(x / scale).clip(-127, 127).astype(np.int8)
    return xi, scale


def _in_maps(x, past_k, past_v, Wq, Wk, Wv, Wo):
    ropes = _rope_consts()
    onesr = np.ones((1, D), np.float32)
    # xR[p, t, b] = x[b, t*128+p]
    xR = np.ascontiguousarray(
        x[:, 0, :].T.reshape(T, 128, B).transpose(1, 0, 2)).astype(NPF16)
    in_maps = []
    for c in range(NCORES):
        wq_c = Wq[c * HQ * D:(c + 1) * HQ * D]             # [512, 4096]
        wk_c = Wk[c * D:(c + 1) * D]                       # [128, 4096]
        wv_c = Wv[c * D:(c + 1) * D]
        wqkvT = np.concatenate([wq_c, wk_c, wv_c], axis=0).T  # [4096, 768]
        wR = np.ascontiguousarray(
            wqkvT.reshape(T, 128, QKV).transpose(1, 0, 2)).astype(NPF16)
        woT = np.ascontiguousarray(
            Wo[:, c * HQ * D:(c + 1) * HQ * D].T).astype(NPF16)
        # K int8: [b, d, s] with scale per (b, d)
        kT_c = np.ascontiguousarray(past_k[:, c].transpose(0, 2, 1))
        kT8, ksc = _quant8(kT_c, axis=2)                   # ksc [B, 128, 1]
        # V int8: vR[b, p, t, d] with scale per (b, p)
        vR = np.ascontiguousarray(
            past_v[:, c].reshape(B, T, 128, D).transpose(0, 2, 1, 3))
        vR8, vsc = _quant8(vR, axis=(2, 3))                # vsc [B, 128, 1, 1]
        cst = np.zeros((128, 21), np.float32)
        cst[:, 0:4] = ropes
        cst[:, 4] = 1.0
        cst[:, 5:13] = ksc[:, :, 0].T                      # [128, B]
        cst[:, 13:21] = vsc[:, :, 0, 0].T
        in_maps.append({
            "xR": xR, "wR": wR, "woT": woT,
            "kT8": np.ascontiguousarray(kT8),
            "vR8": np.ascontiguousarray(vR8),
            "cst": cst, "onesr": onesr,
        })
    return in_maps


def kernel(x, past_k, past_v, Wq, Wk, Wv, Wo):
    assert x.shape == (B, 1, HID) and past_k.shape == (B, NKV, S, D)
    x = np.asarray(x, np.float32)
    past_k = np.asarray(past_k, np.float32)
    past_v = np.asarray(past_v, np.float32)
    Wq = np.asarray(Wq, np.float32)
    Wk = np.asarray(Wk, np.float32)
    Wv = np.asarray(Wv, np.float32)
    Wo = np.asarray(Wo, np.float32)

    if "nc" not in _CACHE:
        _CACHE["nc"] = _build_module()
    nc = _CACHE["nc"]

    in_maps = _in_maps(x, past_k, past_v, Wq, Wk, Wv, Wo)
    res = run_bass_kernel_spmd(nc, in_maps, list(range(NCORES)))
    acc = np.zeros((B, HID), np.float64)
    for c in range(NCORES):
        acc += res.results[c]["o"]
    return acc.astype(np.float32).reshape(B, 1, HID)
